# revision 2
# baseline (speedup 1.0000x reference)
"""BitFFN (ternary-quantized MLP) Trainium2 kernel, data-parallel over 8 NeuronCores.

Pipeline-optimized version (v3) over the original baseline:
  - scale phase split per weight matrix: w1's |w|-sum AllReduce launches after
    reading only w1's slice (half the critical-path DMA); w2's AllReduce is
    issued right after and its latency hides under fc1's matmuls.
  - ternary weights stored as fp8e4 ({-1,0,1} exact): halves w1q/w2q SBUF and
    DRAM traffic. Matmuls run mixed fp8 (stationary) x fp16 (moving).
  - fc2 swapped orientation: out[rows, d_model] = hT.T @ w2qT with kxm=hT,
    kxn=w2qT. w2qT (fp8) is the re-streamed operand -> less DMA than
    re-streaming fp16 hT; output is row-major so the host does no transpose.
  - b2 handled as a [128, D_MODEL] partition-broadcast tile built once via
    K=1 matmuls (free-axis bias).

`repeats` unrolls the whole pipeline N times in one NEFF for marginal-time
measurement; the graded path uses repeats=1.
"""
import os
from contextlib import ExitStack

import numpy as np

import concourse.mybir as mybir
import concourse.tile as tile
from concourse import bacc
from concourse.bass_utils import run_bass_kernel_spmd
from concourse.bass import ds
from concourse.kernels.tile_matmul import (
    TileKxM,
    TileKxN,
    composable_matmul_tile_kernel,
    dma_from_dram_kxm,
    dma_from_dram_kxn,
    dma_to_dram_mxn,
)

F32 = mybir.dt.float32
HALF = mybir.dt.float16  # same PE rate as bf16, 10 mantissa bits
FP8 = mybir.dt.float8e4  # ternary weights are exact in fp8e4
P = 128
D_MODEL = 2048
D_FF = 8192
N_CORES = 8
M_TOTAL = 4 * 4096
M_CORE = M_TOTAL // N_CORES  # 2048 rows per core
N_W = D_FF * D_MODEL  # elements per weight matrix
SLICE_F = N_W // N_CORES // P  # 16384: per-core scale slice is [128, SLICE_F]

GELU = mybir.ActivationFunctionType.Gelu
IS_GE = mybir.AluOpType.is_ge
IS_LE = mybir.AluOpType.is_le
ADD = mybir.AluOpType.add
AX = mybir.AxisListType.X

_BUILD_CACHE = {}


def _emit_scale_chain(nc, tc, wap, const, sstage, dram, thr_pos, thr_neg, tag):
    """|w|-sum over this core's slice -> AllReduce -> [P,1] thresholds.

    Uses only DVE + GPSIMD + tiny SWDGE DMAs: no PE work and no PSUM banks, so
    nothing here conflicts with the matmul pipelines' PSUM pools, and the
    control DMAs bypass the bulk-transfer FIFO on the SP queue.
    """
    from bass_rust import ReduceOp

    CH = 2048
    NCH = SLICE_F // CH  # 8 chunks
    acc = const.tile([P, NCH], F32, tag=f"acc{tag}")
    for i in range(NCH):
        t = sstage.tile([P, CH], F32)
        nc.sync.dma_start(out=t[:], in_=wap.ap()[:, i * CH : (i + 1) * CH])
        nc.vector.tensor_reduce(
            acc[:, i : i + 1], t[:], axis=AX, op=ADD, apply_absolute_value=True
        )
    red = const.tile([P, 1], F32, tag=f"red{tag}")
    nc.vector.tensor_reduce(red[:], acc[:], axis=AX, op=ADD)
    # scale by 0.7/N_W before the reductions so the AllReduce output is
    # directly the ternarization threshold
    nc.vector.tensor_scalar_mul(red[:], red[:], 0.7 / N_W)
    par = const.tile([P, 1], F32, tag=f"par{tag}")
    nc.gpsimd.partition_all_reduce(par[:], red[:], P, ReduceOp.add)

    cc_in = dram.tile([1, 1], F32, tag=f"ccin{tag}")
    cc_out = dram.tile([1, 1], F32, tag=f"ccout{tag}")
    nc.gpsimd.dma_start(out=cc_in[:], in_=par[0:1, :])
    if os.environ.get("BITFFN_NO_CC"):
        # single-core timeline-sim / timing-probe mode: stub the collective
        nc.gpsimd.dma_start(out=cc_out[:], in_=cc_in[:])
    else:
        nc.gpsimd.collective_compute(
            "AllReduce",
            ADD,
            replica_groups=[list(range(N_CORES))],
            ins=[cc_in[:]],
            outs=[cc_out[:]],
        )
    tot_sb = const.tile([1, 2], F32, tag=f"tot{tag}")
    nc.gpsimd.dma_start(out=tot_sb[:, 0:1], in_=cc_out[:])
    nc.vector.tensor_scalar_mul(tot_sb[:, 1:2], tot_sb[:, 0:1], -1.0)
    nc.gpsimd.partition_broadcast(thr_pos[:], tot_sb[0:1, 0:1])
    nc.gpsimd.partition_broadcast(thr_neg[:], tot_sb[0:1, 1:2])


def _emit_pipeline(nc, tc, ios, const, dram, b1_sb, b2bc, rep):
    hT = dram.tile([D_FF, M_CORE], HALF, tag=f"hT{rep}")
    w2qT = dram.tile([D_FF, D_MODEL], FP8, tag=f"w2qT{rep}")

    # ---------------- fc1 (+ interleaved w2 quant) ----------------
    with ExitStack() as fc1_scope:
        sstage = fc1_scope.enter_context(tc.tile_pool(name=f"sstage{rep}", bufs=4))
        sconst = fc1_scope.enter_context(tc.tile_pool(name=f"sconst{rep}", bufs=1))
        t1p = sconst.tile([P, 1], F32, tag="t1p")
        t1n = sconst.tile([P, 1], F32, tag="t1n")
        t2p = sconst.tile([P, 1], F32, tag="t2p")
        t2n = sconst.tile([P, 1], F32, tag="t2n")
        # w1 chain first: its AllReduce gates fc1.  w2's chain is emitted
        # later (from inside fc1's kxm producer) so its slice reads don't
        # precede fc1's first operand DMAs in the queues; its collective
        # latency hides under fc1's matmuls.
        _emit_scale_chain(
            nc, tc, ios["w1s"], sconst, sstage, dram, t1p, t1n, f"w1_{rep}"
        )
        stage = fc1_scope.enter_context(tc.tile_pool(name=f"kxm_stage{rep}", bufs=3))
        kxmq = fc1_scope.enter_context(tc.tile_pool(name=f"kxmq{rep}", bufs=10))
        qtmp = fc1_scope.enter_context(tc.tile_pool(name=f"qtmp{rep}", bufs=3))
        # holds ALL of xT (4 k-tiles x 4 n-tiles of [128,4,512] fp16 = 64KB/par)
        kxn1 = fc1_scope.enter_context(tc.tile_pool(name=f"kxn1{rep}", bufs=16))
        w2stage = fc1_scope.enter_context(tc.tile_pool(name=f"w2stage{rep}", bufs=2))
        w2tmp = fc1_scope.enter_context(tc.tile_pool(name=f"w2tmp{rep}", bufs=2))
        w2qsb = fc1_scope.enter_context(tc.tile_pool(name=f"w2qsb{rep}", bufs=2))

        # w2 ternarization -> DRAM, one block per fc1 kxm-producer call so its
        # elementwise work interleaves with fc1's w1 quant.
        w2_state = {"blk": 0}

        def emit_w2_block():
            if kxm_calls["n"] < 9:
                # t2p/t2n not emitted yet (w2 scale chain lands at call #5);
                # reads before the writer's emission would go untracked.
                return
            blk = w2_state["blk"]
            if blk >= D_FF // P:
                return
            w2_state["blk"] = blk + 1
            s = w2stage.tile([P, D_MODEL], F32)
            nc.sync.dma_start(out=s[:], in_=ios["w2T"].ap()[blk * P : (blk + 1) * P, :])
            q = w2qsb.tile([P, D_MODEL], FP8)
            a = w2tmp.tile([P, D_MODEL], FP8)
            nc.any.tensor_scalar(q[:], s[:], t2p[:, 0:1], None, IS_GE)
            nc.any.tensor_scalar(a[:], s[:], t2n[:, 0:1], None, IS_LE)
            nc.any.tensor_sub(q[:], q[:], a[:])
            nc.sync.dma_start(out=w2qT[blk * P : (blk + 1) * P, :], in_=q[:])

        base_producer, kxm_shape = dma_from_dram_kxm(stage, ios["w1T"].ap())
        base_kxn_producer, kxn_shape = dma_from_dram_kxn(kxn1, ios["xT"].ap())

        # xT is only 8MB/core in fp16 -- memoize tiles so each (k, n) block is
        # DMA'd exactly once and lives in SBUF for all 16 m-stripes.
        xt_memo = {}
        stage_memo = {}
        kxm_calls = {"n": 0}

        def kxm_q_producer(nc_, md):
            key = (md.k_tile_idx, md.m_tile_idx)
            if key in stage_memo:
                t32 = stage_memo.pop(key)
            else:
                t32 = base_producer(nc_, md)
            q = kxmq.tile([P, md.k_subtiles, md.m_tile], FP8, tag="kxmq")
            a = qtmp.tile([P, md.k_subtiles, md.m_tile], FP8, tag="qtmp")
            nc_.any.tensor_scalar(q[:], t32[:], t1p[:, 0:1], None, IS_GE)
            nc_.any.tensor_scalar(a[:], t32[:], t1n[:, 0:1], None, IS_LE)
            nc_.any.tensor_sub(q[:], q[:], a[:])
            kxm_calls["n"] += 1
            if kxm_calls["n"] == 9:
                # m-stripes 0-1's operand DMAs are all emitted by now; the w2
                # scale chain's slice reads queue up behind them.
                _emit_scale_chain(
                    nc, tc, ios["w2s"], sconst, sstage, dram, t2p, t2n, f"w2_{rep}"
                )
            emit_w2_block()
            return q

        def kxn_producer(nc_, md):
            key = (md.k_tile_idx, md.n_tile_idx)
            if key not in xt_memo:
                xt_memo[key] = base_kxn_producer(nc_, md)
            return xt_memo[key]

        # Prewarm fc1's first operands right after the w1 scale reads so the
        # first matmuls aren't stuck behind later bulk DMAs: m-stripe 0's
        # first three w1T k-tiles (stage pool is 3 deep) and x's first
        # n-stripe.
        for kt in range(3):
            stage_memo[(kt, 0)] = base_producer(
                nc,
                TileKxM(
                    k_batch_idx=0, k_tile_idx=kt, k_tile=512, k_subtiles=4,
                    k_subtile=P, m_batch_idx=0, m_tile_idx=0, m_tile=512,
                    m_subtiles=4, m_subtile=P, alloc_shape=None,
                ),
            )
        for kt in range(4):
            xt_memo[(kt, 0)] = base_kxn_producer(
                nc,
                TileKxN(
                    k_batch_idx=0, k_tile_idx=kt, k_tile=512, k_subtiles=4,
                    k_subtile=P, n_batch_idx=0, n_tile_idx=0, n_tile=512,
                    n_subtiles=1, n_subtile=P, alloc_shape=None,
                ),
            )

        def fc1_reducer(nc_, psum, sbuf, md):
            j = md.m_tile_idx * md.m_subtiles + md.m_subtile_idx
            nc_.scalar.activation(sbuf, psum, GELU, bias=b1_sb[:, j : j + 1])

        composable_matmul_tile_kernel(
            tc=tc,
            kxm_shape=kxm_shape,
            kxn_shape=kxn_shape,
            output_type=HALF,
            kxm_producer=kxm_q_producer,
            kxn_producer=kxn_producer,
            mxn_consumer=dma_to_dram_mxn(hT[:]),
            mxn_subtile_reducer=fc1_reducer,
            psum_n_bufs=2,
        )

        # drain any w2 blocks not covered by producer calls
        while w2_state["blk"] < D_FF // P:
            emit_w2_block()

    # ---------------- fc2 (swapped: out[rows, d_model]) ----------------
    with ExitStack() as fc2_scope:
        kxm2 = fc2_scope.enter_context(tc.tile_pool(name=f"kxm2{rep}", bufs=18))
        kxn2 = fc2_scope.enter_context(tc.tile_pool(name=f"kxn2{rep}", bufs=18))

        kxm2_producer, kxm2_shape = dma_from_dram_kxm(kxm2, hT[:])
        kxn2_producer, kxn2_shape = dma_from_dram_kxn(kxn2, w2qT[:])

        def fc2_reducer(nc_, psum, sbuf, md):
            off = md.n_tile_idx * md.n_tile + md.n_subtile_idx * md.n_subtile
            nc_.vector.tensor_add(
                sbuf, psum, b2bc[:, ds(off, md.n_subtile_slice_size)]
            )

        composable_matmul_tile_kernel(
            tc=tc,
            kxm_shape=kxm2_shape,
            kxn_shape=kxn2_shape,
            output_type=F32,
            kxm_producer=kxm2_producer,
            kxn_producer=kxn2_producer,
            mxn_consumer=dma_to_dram_mxn(ios["out"].ap()),
            mxn_subtile_reducer=fc2_reducer,
            psum_n_bufs=2,
        )


def _build_nc(repeats=1):
    nc = bacc.Bacc("TRN2", target_bir_lowering=False, debug=False, num_devices=N_CORES)

    ios = {
        "xT": nc.declare_dram_parameter("xT", [D_MODEL, M_CORE], HALF, isOutput=False),
        "w1T": nc.declare_dram_parameter("w1T", [D_MODEL, D_FF], F32, isOutput=False),
        "w2T": nc.declare_dram_parameter("w2T", [D_FF, D_MODEL], F32, isOutput=False),
        "w1s": nc.declare_dram_parameter("w1s", [P, SLICE_F], F32, isOutput=False),
        "w2s": nc.declare_dram_parameter("w2s", [P, SLICE_F], F32, isOutput=False),
        "b1": nc.declare_dram_parameter("b1", [D_FF], F32, isOutput=False),
        "b2": nc.declare_dram_parameter("b2", [D_MODEL], F32, isOutput=False),
        "out": nc.declare_dram_parameter("out", [M_CORE, D_MODEL], F32, isOutput=True),
    }

    with tile.TileContext(nc) as tc, ExitStack() as top:
        const = top.enter_context(tc.tile_pool(name="const", bufs=1))
        dram = top.enter_context(tc.tile_pool(name="dram", bufs=1, space="DRAM"))

        b1_sb = const.tile([P, D_FF // P], F32)
        nc.scalar.dma_start(
            out=b1_sb[:], in_=ios["b1"].ap().rearrange("(a p) -> p a", p=P)
        )
        # b2 as a [P, D_MODEL] partition-broadcast tile (free-axis bias for the
        # swapped fc2)
        b2row = const.tile([1, D_MODEL], F32)
        nc.scalar.dma_start(
            out=b2row[:], in_=ios["b2"].ap().rearrange("(o d) -> o d", o=1)
        )
        b2bc = const.tile([P, D_MODEL], F32)
        nc.gpsimd.partition_broadcast(b2bc[:], b2row[0:1, :])

        for rep in range(repeats):
            _emit_pipeline(nc, tc, ios, const, dram, b1_sb, b2bc, rep)

    nc.compile()
    return nc


def _get_nc(repeats=1):
    if repeats not in _BUILD_CACHE:
        _BUILD_CACHE[repeats] = _build_nc(repeats)
    return _BUILD_CACHE[repeats]


def _prepare_in_maps(x, w1, b1, w2, b2):
    x = np.asarray(x, dtype=np.float32)
    w1 = np.asarray(w1, dtype=np.float32)
    w2 = np.asarray(w2, dtype=np.float32)
    b1 = np.asarray(b1, dtype=np.float32)
    b2 = np.asarray(b2, dtype=np.float32)

    x2 = x.reshape(M_TOTAL, D_MODEL)
    w1T = np.ascontiguousarray(w1.T)  # [D_MODEL, D_FF] f32
    w2T = np.ascontiguousarray(w2.T)  # [D_FF, D_MODEL] f32
    w1sl = w1.reshape(N_CORES, P, SLICE_F)
    w2sl = w2.reshape(N_CORES, P, SLICE_F)

    in_maps = []
    for c in range(N_CORES):
        shard = x2[c * M_CORE : (c + 1) * M_CORE]
        xT_c = np.ascontiguousarray(shard.T).astype(np.float16)
        in_maps.append(
            {
                "xT": xT_c,
                "w1T": w1T,
                "w2T": w2T,
                "w1s": w1sl[c],
                "w2s": w2sl[c],
                "b1": b1,
                "b2": b2,
            }
        )
    return in_maps


def _assemble(res):
    out = np.concatenate([res.results[c]["out"] for c in range(N_CORES)], axis=0)
    return np.ascontiguousarray(out).reshape(4, 4096, D_MODEL).astype(np.float32, copy=False)


def kernel(x, w1, b1, w2, b2):
    nc = _get_nc()
    in_maps = _prepare_in_maps(x, w1, b1, w2, b2)
    res = run_bass_kernel_spmd(nc, in_maps, list(range(N_CORES)))
    return _assemble(res)


if __name__ == "__main__":
    rng = np.random.default_rng(0)
    x = rng.standard_normal((4, 4096, D_MODEL), dtype=np.float32)
    w1 = rng.standard_normal((D_FF, D_MODEL), dtype=np.float32)
    w2 = rng.standard_normal((D_MODEL, D_FF), dtype=np.float32)
    out = kernel(
        x=x,
        w1=w1,
        b1=np.zeros(D_FF, np.float32),
        w2=w2,
        b2=np.zeros(D_MODEL, np.float32),
    )
    print(out.shape, out.dtype)
